# revision 1
# baseline (speedup 1.0000x reference)
"""Trainium2 Bass kernel for the HFNN (hierarchical fuzzy NN) forward pass.

Math (branch k of 8, rule r of 32, feature f of 16, batch b of 32768):
  expo[k,b,r] = sum_f (x-mu)^2/(2 sigma^2);  E = exp(-expo)
  tsk[k,b]    = sum_r E*(w3b + w3.x) / sum_r E
  out         = softmax over 2 classes of w5 @ tsk + b5

Design (pure batch data-parallel over 8 cores, 4096 batch each, chunked):
  - Host ships an fp16 "ab" tensor ([wq|bias] head + per-chunk [x^2;x]
    slabs for branch groups 0-3 / 4-7) and an fp16-container "cw" tensor
    (bf16 post-exp weights + scatter idxs + fp16 C=x slab). Membership
    is ONE K=128 fp16 matmul per group (quad+lin merged); pre-exp must
    be fp16 (bf16 fails the tolerance: errors exponentiate).
  - One wide ACT exp per chunk over a bank-aligned [128,2,512] PSUM tile
    with per-partition bias b[p]=max over the two groups of c[k,r]; the
    per-rule exp(b-c) factor is folded into all downstream bf16 lhsT
    weights (needs bf16 exponent range).
  - G association: G[k,f,b] = sum_r w3~E'; xg = C*G on DVE (bf16 out:
    den can be ~1e-11, fp16 would flush); reductions accumulate den and
    num (=numb+num0, folded in PSUM) into [32,w] collectors per 2-chunk
    round.
  - ~50 dummy matmuls warm the PE p-state ramp during the input-DMA
    window; the ACT table load is pulled off the critical path by a
    1-element warm-up exp.
  - Rounds 0-2 exit via staged SBUF copies + HWDGE DMAs; the last two
    rounds are staged into one tile and leave via a PREPARED SWDGE
    scatter-add (descriptors generated early; trigger_dma after the last
    copy) into a pre-zeroed output, skipping the HWDGE gen + DGE delay
    in the kernel tail. A post-finalize patch points the descriptor sem
    at the DMASW lane the Tile framework's final drain expects.
  - Host epilogue exact in float64: tsk=num/den, d=(w5d).tsk+b5d,
    p=sigmoid(+-d).
"""

import numpy as np
import ml_dtypes

import concourse.bacc as bacc
import concourse.tile as tile
from concourse import mybir
from concourse.bass_utils import run_bass_kernel_spmd

F32 = mybir.dt.float32
F16 = mybir.dt.float16
BF16 = mybir.dt.bfloat16

NB, NR, NF = 8, 32, 16
NBATCH, NCORE = 32768, 8
BC = NBATCH // NCORE          # 4096 batch per core

CHUNKS = [256, 256] + [512] * 6 + [384, 128]
OFFS = np.concatenate([[0], np.cumsum(CHUNKS)]).tolist()
NCH = len(CHUNKS)
ROUNDS = [(0, 1), (2, 3), (4, 5), (6, 7), (8, 9)]
NRND = len(ROUNDS)
NWARM = 50

# ab tensor: [wq(256) | bias(2) | chunk0 A|B (2*W0) | chunk1 A|B ...]
AB_W = 258 + 2 * BC
# cw tensor: [wg(256) | wred(128) | wx(64) | scatter idxs(2) | C chunks...]
CW_W = 450 + BC
ZW = 896                      # combined scatter output width (r3 512 + r4 384)

_CACHE: dict = {}


def _build_nc():
    nc = bacc.Bacc("TRN2", target_bir_lowering=False, debug=False)
    ab_in = nc.dram_tensor("ab", [128, AB_W], F16, kind="ExternalInput")
    cw_in = nc.dram_tensor("cw", [128, CW_W], F16, kind="ExternalInput")
    out_c = nc.dram_tensor("outc", [3, 32, 512], F32, kind="ExternalOutput")
    out_z = nc.dram_tensor("outz", [32, ZW], BF16, kind="ExternalOutput")

    # input DMA pieces, in chunk index ranges
    PIECES_AB = [(0, 1), (1, 2), (2, 3), (3, 4), (4, 5), (5, 6), (6, 7),
                 (7, 8), (8, 10)]
    PIECES_C = [(0, 2), (2, 4), (4, 6), (6, 8), (8, 10)]

    with tile.TileContext(nc) as tc:
        with (
            tc.tile_pool(name="wpool", bufs=1) as wpool,
            tc.tile_pool(name="apool", bufs=4) as apool,
            tc.tile_pool(name="cpool", bufs=4) as cpool,
            tc.tile_pool(name="epool", bufs=3) as epool,
            tc.tile_pool(name="xpool", bufs=3) as xpool,
            tc.tile_pool(name="opool", bufs=5) as opool,
            tc.tile_pool(name="mps", bufs=2, space="PSUM") as mps,
            tc.tile_pool(name="gps", bufs=2, space="PSUM") as gpsp,
            tc.tile_pool(name="collps", bufs=2, space="PSUM") as collps,
        ):
            # --- warm-up: ACT table load + PE p-state ramp, off critical path
            wz = wpool.tile([128, 64], BF16, tag="wz")
            nc.vector.memset(wz[:], 0.25)
            warme = wpool.tile([128, 1], F32, tag="warme")
            nc.vector.memset(warme[:], 0.0)
            nc.scalar.activation(warme[:], warme[:],
                                 mybir.ActivationFunctionType.Exp)
            wps = collps.tile([32, 512], F32, tag="coll", name="warm")
            for _ in range(NWARM):
                nc.tensor.matmul(wps[:, 0:64], wz[:, 0:32], wz[:],
                                 start=True, stop=True)

            # --- post-exp weights: two pieces on the gpsimd/SWDGE queue ---
            wct = wpool.tile([128, 450], F16, tag="wct")
            wcb = wct.bitcast(BF16)
            # combined staging + zeroed dram for the prepared scatter-DMA
            # (final two rounds bypass the HWDGE gen + DGE delay in the tail)
            otc = wpool.tile([128, ZW], BF16, tag="otc")
            nc.vector.memset(otc[:], 0.0)
            zot = wpool.tile([32, ZW], BF16, tag="zot")
            nc.vector.memset(zot[:], 0.0)
            dma_sem = nc.alloc_semaphore("sc_dma")

            def wg(g):
                return wcb[:, 128 * g: 128 * (g + 1)]

            def wred(g, q):
                i = 2 * q + g
                return wcb[:, 256 + 32 * i: 256 + 32 * (i + 1)]

            def wx(q):
                return wcb[:, 384 + 32 * q: 384 + 32 * (q + 1)]

            # --- input slab DMAs; head piece + weights on gpsimd (short
            # preamble), the rest on SP/HWDGE in SCHED order ---
            abt, ct = {}, {}

            def emit_ab(pi, eng):
                lo, hi = PIECES_AB[pi]
                o0, o1 = OFFS[lo], OFFS[hi]
                w2 = 2 * (o1 - o0)
                if pi == 0:
                    hd = wpool.tile([128, 258 + w2], F16, tag="hd")
                    eng.dma_start(out=hd[:], in_=ab_in[:, 0:258 + w2])
                    a_t = hd[:, 258:]
                    res = (hd[:, 0:256], hd[:, 256:258].bitcast(F32))
                else:
                    a_tile = apool.tile([128, w2], F16, tag="a", name=f"a{lo}")
                    eng.dma_start(
                        out=a_tile[:],
                        in_=ab_in[:, 258 + 2 * o0: 258 + 2 * o1])
                    a_t = a_tile[:]
                    res = None
                for c in range(lo, hi):
                    sl0 = 2 * (OFFS[c] - o0)
                    abt[c] = a_t[:, sl0: sl0 + 2 * CHUNKS[c]]
                return res

            def emit_c(pi):
                lo, hi = PIECES_C[pi]
                o0, o1 = OFFS[lo], OFFS[hi]
                c_t = cpool.tile([128, o1 - o0], F16, tag="c", name=f"c{lo}")
                nc.sync.dma_start(
                    out=c_t[:], in_=cw_in[:, 450 + o0: 450 + o1])
                for c in range(lo, hi):
                    ct[c] = c_t[:, OFFS[c] - o0: OFFS[c] - o0 + CHUNKS[c]]

            wq, bias_t = emit_ab(0, nc.sync)
            nc.gpsimd.dma_start(out=wct[:, 0:256], in_=cw_in[:, 0:256])
            nc.gpsimd.dma_start(out=wct[:, 256:450], in_=cw_in[:, 256:450])
            nc.gpsimd.dma_start(out=out_z[:, :], in_=zot[:])
            idxs = wct[:, 448:450].bitcast(mybir.dt.int16)
            nc.gpsimd.dma_scatter_add(
                out_z[:, :], otc[:].unsqueeze(1), idxs, 32, 32, ZW,
                prepare_only=True, sem=dma_sem,
            )
            SCHED = [("ab", 1), ("ab", 2), ("c", 0), ("ab", 3), ("ab", 4),
                     ("c", 1), ("ab", 5), ("c", 2), ("ab", 6), ("c", 3),
                     ("ab", 7), ("ab", 8), ("c", 4)]
            for kind, pi in SCHED:
                if kind == "ab":
                    emit_ab(pi, nc.sync)
                else:
                    emit_c(pi)

            # --- software-pipelined chunk loop ---
            m2w, e_t, gx, xg_t, coll, ot = {}, {}, {}, {}, {}, {}
            rnd_of = {}
            for r, (c0, c1) in enumerate(ROUNDS):
                rnd_of[c0] = (r, 0)
                rnd_of[c1] = (r, 1)
            for c in range(NCH + 2):
                if c < NCH:
                    w = CHUNKS[c]
                    m2w[c] = mps.tile([128, 2, 512], F32, tag="m",
                                      name=f"m{c}")
                    for g in range(2):
                        nc.tensor.matmul(
                            m2w[c][:, g, 0:w],
                            wq[:, 128 * g:128 * (g + 1)],
                            abt[c][:, g * w:(g + 1) * w],
                            start=True, stop=True,
                        )
                    e_t[c] = epool.tile([128, 2 * w], BF16, tag="e",
                                        name=f"e{c}")
                    nc.scalar.activation(
                        e_t[c][:], m2w[c][:, :, 0:w],
                        mybir.ActivationFunctionType.Exp,
                        bias=bias_t[:, 0:1], scale=1.0,
                    )
                d = c - 1
                if 0 <= d < NCH:
                    w = CHUNKS[d]
                    gx[d] = gpsp.tile([128, w], F32, tag="g", name=f"g{d}")
                    for g in range(2):
                        nc.tensor.matmul(
                            gx[d][:], wg(g), e_t[d][:, g * w:(g + 1) * w],
                            start=(g == 0), stop=(g == 1),
                        )
                    xg_t[d] = xpool.tile([128, w], BF16, tag="xg",
                                         name=f"xg{d}")
                    nc.vector.tensor_mul(xg_t[d][:], ct[d], gx[d][:])
                    r, q = rnd_of[d]
                    if q == 0:
                        wr = max(CHUNKS[cc] for cc in ROUNDS[r])
                        coll[r] = collps.tile([32, wr], F32, tag="coll",
                                              name=f"coll{r}")
                    for g in range(2):
                        nc.tensor.matmul(
                            coll[r][:, 0:w], wred(g, q),
                            e_t[d][:, g * w:(g + 1) * w],
                            start=(q == 0 and g == 0), stop=False,
                        )
                d = c - 2
                if 0 <= d < NCH:
                    w = CHUNKS[d]
                    r, q = rnd_of[d]
                    nc.tensor.matmul(
                        coll[r][:, 0:w], wx(q), xg_t[d][:],
                        start=False, stop=(q == 1),
                    )
                    if q == 1:
                        wr = coll[r].shape[1]
                        if r < 3:
                            ot[r] = opool.tile([32, wr], F32, tag="o",
                                               name=f"o{r}")
                            nc.vector.tensor_copy(ot[r][:], coll[r][:])
                        elif r == 3:
                            nc.scalar.copy(otc[0:32, 0:wr], coll[r][:])
                        else:
                            nc.scalar.copy(otc[0:32, 512:512 + wr],
                                           coll[r][:])
            for r in range(3):
                wr = ot[r].shape[1]
                nc.sync.dma_start(out=out_c[r][:, 0:wr], in_=ot[r][:])
            nc.gpsimd.trigger_dma(count=None)
    nc.finalize()
    _patch_prep_dmasw(nc)
    return nc


def _patch_prep_dmasw(nc):
    """Tile assigns the prepared scatter a DMASW lane and makes the final
    drain wait on it, but never attaches the lane increment to any
    instruction (the prep's descriptor sem is the user sem). Attach the
    missing update to the prep so the kernel can drain; the DMA's real
    completion is tracked by the descriptor-embedded user sem."""
    waited, updated, preps = {}, set(), []
    for fn in nc.m.functions:
        for blk in fn.blocks:
            for inst in blk.instructions:
                si = inst.sync_info
                if si is None:
                    continue
                for w in si.on_wait:
                    if w.ant_name and w.ant_name.startswith("DMASW"):
                        waited[w.id] = (w.ant_name, w.wait_value)
                for u in si.on_update:
                    updated.add(u.id)
                if type(inst).__name__ == "InstDMAScatterAddAnt":
                    preps.append(inst)
    missing = [(i, nm, val) for i, (nm, val) in waited.items()
               if i not in updated]
    assert len(missing) == len(preps) == 1, (missing, len(preps))
    i, nm, val = missing[0]
    si = preps[0].sync_info
    su0 = si.on_update[0]
    nu = type(su0)(sync_type="semaphore", id=i, ant_name=nm,
                   update_mode="sem-add-imm", update_value=val,
                   update_reg=None)
    upd = si.on_update
    upd[0] = nu         # descriptor sem -> the framework's DMASW lane
    si.on_update = upd


def _host_prep(para_mu, para_sigma, para_w3):
    mu = np.float64(para_mu)
    sig = np.float64(para_sigma)
    w3 = np.float64(para_w3)
    a_neg = -1.0 / (2 * sig * sig)
    m2 = mu / (sig * sig)
    c = np.sum(mu * mu / (2 * sig * sig), axis=-1)      # [8, 32]

    p_i = np.repeat(np.arange(4), NR)
    p_r = np.tile(np.arange(NR), 4)
    bmax = np.float64(np.float32(np.maximum(c[p_i, p_r], c[4 + p_i, p_r])))
    scale = np.stack([np.exp(bmax - c[4 * g + p_i, p_r]) for g in range(2)])

    wq = np.zeros((128, 256), np.float64)
    wgm = np.zeros((128, 256), np.float64)
    wrd = np.zeros((128, 128), np.float64)
    wxr = np.zeros((128, 64), np.float64)
    for g in range(2):
        for i in range(4):
            k = 4 * g + i
            cols = slice(32 * i, 32 * i + 32)
            wq[16 * i:16 * i + 16, 128 * g + 32 * i:128 * g + 32 * i + 32] = \
                a_neg[k].T
            wq[64 + 16 * i:64 + 16 * i + 16,
               128 * g + 32 * i:128 * g + 32 * i + 32] = m2[k].T
            wgm[cols, 128 * g + 64 * g + 16 * i:
                128 * g + 64 * g + 16 * i + 16] = \
                w3[k, :, :NF] * scale[g][cols, None]
            for q in range(2):
                blk = 32 * (2 * q + g)
                wrd[cols, blk + 16 * q + k] = scale[g][cols]
                wrd[cols, blk + 16 * q + 8 + k] = w3[k, :, NF] * scale[g][cols]
    for q in range(2):
        for k in range(NB):
            wxr[16 * k:16 * k + 16, 32 * q + 16 * q + 8 + k] = 1.0

    whead = np.zeros((128, 258), np.uint16)
    whead[:, 0:256] = np.float16(wq).view(np.uint16)
    whead[:, 256:258] = np.float32(-bmax)[:, None].view(np.uint16).reshape(
        128, 2)
    wcw = np.zeros((128, 450), np.uint16)
    wcw[:, 0:256] = np.asarray(wgm, np.float32).astype(
        ml_dtypes.bfloat16).view(np.uint16)
    wcw[:, 256:384] = np.asarray(wrd, np.float32).astype(
        ml_dtypes.bfloat16).view(np.uint16)
    wcw[:, 384:448] = np.asarray(wxr, np.float32).astype(
        ml_dtypes.bfloat16).view(np.uint16)
    # scatter idxs: token t = partition row t -> dest row t; [16, 2] wrap
    idx = np.arange(32, dtype=np.int16).reshape(2, 16).T  # [p, j] = 16j + p
    wcw[0:16, 448:450] = idx.view(np.uint16)
    return whead.view(np.float16), wcw.view(np.float16)


def kernel(data, para_mu, para_sigma, para_w3, w5, b5):
    if "nc" not in _CACHE:
        _CACHE["nc"] = _build_nc()
    nc = _CACHE["nc"]

    whead, wcw = _host_prep(para_mu, para_sigma, para_w3)
    xt = np.ascontiguousarray(np.float32(data).transpose(0, 2, 1))  # [8,16,B]
    x16 = np.float16(xt)
    x2 = np.float16(xt.astype(np.float64) ** 2)

    in_maps = []
    for cidx in range(NCORE):
        cols = slice(cidx * BC, (cidx + 1) * BC)
        ab = np.empty((128, AB_W), np.float16)
        ab[:, 0:258] = whead
        A2 = x2[:, :, cols]
        X = x16[:, :, cols]
        for ci, w in enumerate(CHUNKS):
            o = OFFS[ci]
            base = 258 + 2 * o
            csl = slice(o, o + w)
            ab[0:64, base:base + w] = A2[0:4, :, csl].reshape(64, w)
            ab[64:128, base:base + w] = X[0:4, :, csl].reshape(64, w)
            ab[0:64, base + w:base + 2 * w] = A2[4:8, :, csl].reshape(64, w)
            ab[64:128, base + w:base + 2 * w] = X[4:8, :, csl].reshape(64, w)
        cw = np.empty((128, CW_W), np.float16)
        cw[:, 0:450] = wcw
        cw[:, 450:] = X.reshape(128, BC)
        in_maps.append({"ab": ab, "cw": cw})
    try:
        res = run_bass_kernel_spmd(nc, in_maps, core_ids=list(range(NCORE)))
    except Exception:
        res = run_bass_kernel_spmd(nc, in_maps, core_ids=list(range(NCORE)))
    _CACHE["last_result"] = res

    den = np.empty((NB, NBATCH), np.float64)
    num = np.empty((NB, NBATCH), np.float64)
    for cidx in range(NCORE):
        arr = res.results[cidx]["outc"].astype(np.float64)  # [3, 32, 512]
        arrz = res.results[cidx]["outz"].astype(np.float64)  # [32, ZW]
        cols0 = cidx * BC
        for r, (c0, c1) in enumerate(ROUNDS):
            if r < 3:
                blk, off = arr[r], 0
            else:
                blk, off = arrz, 512 * (r - 3)
            for q, ci in enumerate(((c0, c1))):
                w = CHUNKS[ci]
                sl = slice(cols0 + OFFS[ci], cols0 + OFFS[ci] + w)
                den[:, sl] = blk[16 * q:16 * q + 8, off:off + w]
                num[:, sl] = blk[16 * q + 8:16 * q + 16, off:off + w]

    tsk = num / den
    w5d = (w5[0] - w5[1]).astype(np.float64)
    d = w5d @ tsk + (float(b5[0]) - float(b5[1]))
    p0 = 1.0 / (1.0 + np.exp(-d))
    out = np.empty((NBATCH, 2), np.float32)
    out[:, 0] = p0.astype(np.float32)
    out[:, 1] = (1.0 - p0).astype(np.float32)
    return out

